# revision 48
# baseline (speedup 1.0000x reference)
"""Trainium2 Bass kernel for nn_Decay2D (decay-masked linear attention).

Math: the reference's Hillis-Steele scan with decay-squaring order composes
to coefficient d^ceil((t-s)/2) on store[s] = scale*k_s v_s^T, so

    out[t] = scale^2 * sum_{s<=t} d^ceil((t-s)/2) (q_t . k_s) v_s  @ Wo^T

computed as chunked linear attention with a [K, 2V] carry state per chunk
(even/odd decay chains on the V axis), never materializing [B,T,K,V].

Sharding: 8 cores = 4 batches x 2 sequence halves. Each core builds the
carry state over a truncated 128-row prefix and runs full attention +
output projection for its own 1024 rows.

v11 (~44.5us vs 50.2us v3 baseline): the body is built around the DMA
stream. All transfers are contiguous DRAM blocks (header with
gamma/weights/idents/wrep, prefix block, per-(group,half) x blocks, mloc,
a 64-row ce/co/Wo block, per-chunk output stores) on the sync HWDGE ring
in consumption order; the PE chases the stream. ~50 warm-up matmuls on a
memset scratch run while the header streams in so HAM reaches K=8/8
before real work. PSUM rings are split so the attention accumulators
(plt) ping-pong independently of the projection/score/out-proj ring. The
carry chain is bf16 end-to-end (one DVE stt per chunk, no casts), and the
parity weights ride a header-shipped broadcast block (wrep) because both
DVE and POOL tensor_scalar are slow paths (~4x and ~13x vs tensor_mul).
"""

from contextlib import ExitStack

import numpy as np

import concourse.bass as bass
import concourse.bacc as bacc
import concourse.mybir as mybir
import concourse.tile as tile
from concourse import bass_utils
from concourse.alu_op_type import AluOpType
from concourse.bass import ts

F32 = mybir.dt.float32
BF16 = mybir.dt.bfloat16
SIG = mybir.ActivationFunctionType.Sigmoid

B, T, E, K, V = 4, 2048, 1024, 64, 64
DECAY = 0.9
C = 128          # chunk length
HT = T // 2      # rows per core (sequence half)
NCH = HT // C    # chunks per half (8)
NEC = E // 128   # embed sub-chunks (8)
GW = 512         # group width: 4 chunks per PSUM bank
GCH = GW // C    # chunks per group (4)
NG = HT // GW    # groups per half (2)
HW2 = GW // 2    # half-group width (256)
HCOL = NEC * HW2 # x columns per (group, half) block (2048)
DC2 = float(DECAY ** (C // 2))
N_CORES = 8
PRE = 128        # truncated prefix length (1 chunk; older rows decay < 2e-3)

def _mklayout(regions):
    out, off = {}, 0
    for n, r, c in regions:
        out[n] = (r, off, c)
        off += c
    return out, off


# gamma columns (bf16, inside hdr): 0 prefix flag, 1 bk|bv, 2 bq, 3 wge, 4 wgo
G_GAMMA, G_BKV, G_BQ, G_WGE, G_WGO = 0, 1, 2, 3, 4
G_NCOL = 8

_HDR, HDR_W = _mklayout([
    ("gamma", 128, G_NCOL),
    ("wkv", 128, NEC * 2 * K), ("wq", 128, NEC * K),
    ("ident", 64, 64), ("identhi", 128, 64),
    ("wrep", 128, 2 * GCH * V),
])
_CWB, CWB_W = _mklayout([
    ("cemat", K, GW), ("comat", K, GW),
])


def _host_constants():
    d = DECAY
    scale2 = 1.0 - d
    i = np.arange(C)
    j = np.arange(C)
    delta = i[:, None] - j[None, :]
    # intra-chunk decay mask, transposed to [tcol(j), trow(i)], scale^2 folded
    mloc = np.where(delta >= 0, d ** np.ceil(delta / 2.0), 0.0) * scale2
    mlocT4 = np.tile(np.ascontiguousarray(mloc.T), (1, GCH)).astype(np.float32)
    # boundary coefficient per local row i (scale^2 folded), split by parity
    c = d ** np.ceil((i + 1) / 2.0) * scale2
    ce = np.where(i % 2 == 0, c, 0.0).astype(np.float32)
    co = np.where(i % 2 == 1, c, 0.0).astype(np.float32)
    cemat = np.tile(np.broadcast_to(ce, (K, C)), (1, GCH)).astype(np.float32)
    comat = np.tile(np.broadcast_to(co, (K, C)), (1, GCH)).astype(np.float32)
    # state-update row weights (per t within chunk)
    u_o = np.where(j % 2 == 1, d ** ((C - 1 - j) / 2.0), 0.0)
    u_e = np.where(j % 2 == 0, d ** ((C - 2 - j) / 2.0), 0.0)
    wge = (u_o + u_e).astype(np.float32)          # [C]
    wgo = (u_o + d * u_e).astype(np.float32)
    return {
        "mlocT4": mlocT4,
        "cemat": np.ascontiguousarray(cemat),
        "comat": np.ascontiguousarray(comat),
        "wge": wge,
        "wgo": wgo,
        "ident64": np.eye(64, dtype=np.float32),
    }


def _build_program(has_bv):
    nc = bacc.Bacc(
        "TRN2",
        debug=False,
        enable_asserts=False,
        target_bir_lowering=False,
        num_devices=N_CORES,
    )

    def din(name, shape, dtype=BF16):
        return nc.dram_tensor(name, shape, dtype, kind="ExternalInput").ap()

    hdr_d = din("hdr", [128, HDR_W])
    xpre_d = din("xpre", [128, NEC * PRE])        # prefix x, (ec, t)
    xq4 = din("xq4", [NG * 2, 128, HCOL])         # x per (group, half), (ec, t)
    mloc_d = din("mloc", [C, GW])
    cwb_d = din("cwb", [K, CWB_W])                # ce / co (64 rows)
    wo2_d = din("wo2", [128, E])                  # Wo^T duplicated in both halves
    out_d = nc.dram_tensor("out", [NCH, 128, E], BF16,
                           kind="ExternalOutput").ap()

    with ExitStack() as ctx:
        tc = ctx.enter_context(tile.TileContext(nc))

        consts = ctx.enter_context(tc.tile_pool(name="consts", bufs=1))
        state = ctx.enter_context(tc.tile_pool(name="state", bufs=1))
        xpool = ctx.enter_context(tc.tile_pool(name="xg", bufs=2))
        spool = ctx.enter_context(tc.tile_pool(name="sml", bufs=2))
        opool = ctx.enter_context(tc.tile_pool(name="osb", bufs=3))
        # PSUM budget (8 banks): pmain ring 2 (pkv/pq/ps/po), psm2 ring 2
        # (pkv1/pu1/plt ping-pong), psml ring 2 (transposes), pstate 2 (pu2)
        pmain = ctx.enter_context(tc.tile_pool(name="pmain", bufs=2, space="PSUM"))
        psm2 = ctx.enter_context(tc.tile_pool(name="psm2", bufs=2, space="PSUM"))
        psml = ctx.enter_context(tc.tile_pool(name="psml", bufs=2, space="PSUM"))
        pstate = ctx.enter_context(tc.tile_pool(name="pstate", bufs=1, space="PSUM"))

        # ---- loads in wire order (single sync HWDGE ring = FIFO) ----
        hdr = consts.tile([128, HDR_W], BF16, name="hdr")
        nc.sync.dma_start(hdr[:], hdr_d[:])
        xp = consts.tile([128, NEC * PRE], BF16, name="xp")
        nc.sync.dma_start(xp[:], xpre_d[:])

        xg2s = [xpool.tile([128, 2 * HCOL], BF16, tag="xg", name=f"xg2_{g}")
                for g in range(NG)]

        def load_xhalf(g, hf):
            nc.sync.dma_start(
                xg2s[g][:, hf * HCOL : (hf + 1) * HCOL], xq4[g * 2 + hf])

        load_xhalf(0, 0)
        load_xhalf(0, 1)
        load_xhalf(1, 0)
        load_xhalf(1, 1)
        mlocT4 = consts.tile([C, GW], BF16, name="mloc")
        nc.sync.dma_start(mlocT4[:], mloc_d[:])
        cwb = consts.tile([K, CWB_W], BF16, name="cwb")
        nc.sync.dma_start(cwb[:], cwb_d[:])
        wo2 = consts.tile([128, E], BF16, name="wo2")
        nc.sync.dma_start(wo2[:], wo2_d[:])

        def reg(pack, layout, name):
            r, o, c = layout[name]
            return pack[0:r, o : o + c]

        gamma = consts.tile([128, G_NCOL], F32, name="gamma_f32")
        nc.vector.tensor_copy(gamma[:], reg(hdr, _HDR, "gamma"))
        wkv, wq = reg(hdr, _HDR, "wkv"), reg(hdr, _HDR, "wq")
        ident, identhi = reg(hdr, _HDR, "ident"), reg(hdr, _HDR, "identhi")
        cemat, comat = reg(cwb, _CWB, "cemat"), reg(cwb, _CWB, "comat")
        bk_ap = gamma[0:K, G_BKV : G_BKV + 1]
        bv_ap = gamma[K : 2 * K, G_BKV : G_BKV + 1]
        bq_ap = gamma[0:K, G_BQ : G_BQ + 1]
        wge_ap = gamma[:, G_WGE : G_WGE + 1]
        wgo_ap = gamma[:, G_WGO : G_WGO + 1]

        qT_all = consts.tile([K, HT], BF16, name="qT_all")
        kT_all = consts.tile([K, HT], BF16, name="kT_all")
        # lt2 holds chunk pairs: even chunk at partitions 0:64, odd at
        # 64:128 (written there by col-tiled attention), so the two
        # out-projection matmuls of a pair run as concurrent row tiles
        lt2 = consts.tile([128, (NCH // 2) * C], BF16, name="lt2")
        # carry state: parity on the V axis -> [K, NCH * 2V], kept in bf16
        # (the 7-step chain loses ~0.3% which is well inside tolerance)
        geo_bf = state.tile([K, NCH * 2 * V], BF16, name="geo_bf")
        # wge/wgo broadcast into [C, GCH*V] blocks, shipped in the header
        wrep = reg(hdr, _HDR, "wrep")

        def ts2(i):  # [K, 2V] slice of the state for chunk i
            return slice(i * 2 * V, (i + 1) * 2 * V)

        # ---- stage helpers ----
        def transposes(kT_src, vT_src, nch, tagp):
            pkn = psml.tile([C, nch * K], BF16, tag="pS", name=f"pkn{tagp}")
            for cl in range(nch):
                nc.tensor.matmul(pkn[:, ts(cl, K)], kT_src[:, ts(cl, C)],
                                 ident[:], is_transpose=True)
            pvn = psml.tile([C, nch * V], BF16, tag="pS", name=f"pvn{tagp}")
            for cl in range(nch):
                nc.tensor.matmul(pvn[:, ts(cl, V)], vT_src[K : 2 * K, ts(cl, C)],
                                 identhi[K : 2 * K, :], is_transpose=True)
            kn = spool.tile([C, nch * K], BF16, tag=f"kn{tagp}", name=f"kn{tagp}")
            nc.scalar.copy(kn[:], pkn[:])
            v_b = spool.tile([C, nch * V], BF16, tag=f"v{tagp}", name=f"v{tagp}")
            nc.vector.tensor_copy(v_b[:], pvn[:])
            # parity-weighted v for the state update
            veo = spool.tile([C, 2 * nch * V], BF16, tag=f"veo{tagp}",
                             name=f"veo{tagp}")
            nc.vector.tensor_mul(veo[:, 0 : nch * V], v_b[:],
                                 wrep[:, 0 : nch * V])
            nc.vector.tensor_mul(veo[:, nch * V :], v_b[:],
                                 wrep[:, GCH * V : (GCH + nch) * V])
            return kn, v_b, veo

        def scores_stage(g):
            ps = pmain.tile([C, GW], F32, tag="pM", name=f"ps{g}")
            for cl in range(GCH):
                i = g * GCH + cl
                nc.tensor.matmul(ps[:, ts(cl, C)], kT_all[:, ts(i, C)],
                                 qT_all[:, ts(i, C)], start=True, stop=True)
            sT_b = spool.tile([C, GW], BF16, tag=f"sm{g}", name="sT_b")
            nc.vector.tensor_mul(sT_b[:], ps[:], mlocT4[:])
            qTe = spool.tile([K, GW], BF16, tag=f"qe{g}", name="qTe")
            nc.vector.tensor_mul(qTe[:], qT_all[:, ts(g, GW)], cemat[:])
            qTo = spool.tile([K, GW], BF16, tag=f"qo{g}", name="qTo")
            nc.gpsimd.tensor_mul(qTo[:], qT_all[:, ts(g, GW)], comat[:])
            return sT_b, qTe, qTo

        def state_stage(g, kn_g, veo_g):
            # pu2 updates for this group's chunks (skip the last chunk)
            for cl in range(GCH):
                i = g * GCH + cl
                if i < NCH - 1:
                    nc.tensor.matmul(pu2[:, i * 2 * V : i * 2 * V + V],
                                     kn_g[:, ts(cl, K)],
                                     veo_g[:, ts(cl, V)],
                                     start=True, stop=True)
                    nc.tensor.matmul(pu2[:, i * 2 * V + V : (i + 1) * 2 * V],
                                     kn_g[:, ts(cl, K)],
                                     veo_g[:, (GCH + cl) * V : (GCH + cl + 1) * V],
                                     start=True, stop=True)

        def chain_step(i):
            nc.vector.scalar_tensor_tensor(
                geo_bf[:, ts2(i)], geo_bf[:, ts2(i - 1)], DC2,
                pu2[:, ts2(i - 1)], AluOpType.mult, AluOpType.add,
            )

        def attn_out_pair(g, cl, v_b, sT_b, qTe, qTo):
            # chunks a=cl, b=cl+1 of group g; b's attention runs on array
            # columns 64:127 (tile_position) so the pair's matmuls overlap,
            # and its lt lands at partitions 64:128 for row-tiled out-proj
            ia, ib = g * GCH + cl, g * GCH + cl + 1
            j = ia // 2
            pla = psm2.tile([128, C], F32, tag="p2", name=f"plt{ia}")
            plb = psm2.tile([128, C], F32, tag="p2", name=f"plt{ib}")
            nc.tensor.matmul(pla[0:V, :], v_b[:, ts(cl, V)], sT_b[:, ts(cl, C)],
                             start=True, stop=False)
            nc.tensor.matmul(pla[0:V, :], geo_bf[:, ia * 2 * V : ia * 2 * V + V],
                             qTe[:, ts(cl, C)], start=False, stop=False)
            nc.tensor.matmul(pla[0:V, :],
                             geo_bf[:, ia * 2 * V + V : (ia + 1) * 2 * V],
                             qTo[:, ts(cl, C)], start=False, stop=True)
            cb = cl + 1
            nc.tensor.matmul(plb[64:128, :], v_b[:, ts(cb, V)],
                             sT_b[:, ts(cb, C)],
                             start=True, stop=False, tile_position=(0, 64))
            nc.tensor.matmul(plb[64:128, :],
                             geo_bf[:, ib * 2 * V : ib * 2 * V + V],
                             qTe[:, ts(cb, C)],
                             start=False, stop=False, tile_position=(0, 64))
            nc.tensor.matmul(plb[64:128, :],
                             geo_bf[:, ib * 2 * V + V : (ib + 1) * 2 * V],
                             qTo[:, ts(cb, C)],
                             start=False, stop=True, tile_position=(0, 64))
            nc.scalar.copy(lt2[0:64, ts(j, C)], pla[0:V, :])
            nc.vector.tensor_copy(lt2[64:128, ts(j, C)], plb[64:128, :])
            out_sa = opool.tile([C, E], BF16, tag="osb", name=f"out_sb{ia}")
            out_sbb = opool.tile([C, E], BF16, tag="osb", name=f"out_sb{ib}")
            for h in range(2):
                poa = pmain.tile([C, GW], F32, tag="pM", name=f"po{ia}_{h}")
                nc.tensor.matmul(poa[:], lt2[0:64, ts(j, C)],
                                 wo2[0:64, ts(h, GW)], start=True, stop=True)
                pob = pmain.tile([C, GW], F32, tag="pM", name=f"po{ib}_{h}")
                nc.tensor.matmul(pob[:], lt2[64:128, ts(j, C)],
                                 wo2[64:128, ts(h, GW)], start=True, stop=True)
                if h == 0:
                    nc.scalar.copy(out_sa[:, ts(h, GW)], poa[:])
                    nc.vector.tensor_copy(out_sbb[:, ts(h, GW)], pob[:])
                else:
                    nc.vector.tensor_copy(out_sa[:, ts(h, GW)], poa[:])
                    nc.scalar.copy(out_sbb[:, ts(h, GW)], pob[:])
            nc.sync.dma_start(out_d[ia], out_sa[:])
            nc.sync.dma_start(out_d[ib], out_sbb[:])

        # ============ PE warm-up ============
        # ~35 filler matmuls on a memset scratch while the header streams
        # in: HAM flips to K=8/8 (~3.4us of sustained PE busy) so the real
        # matmul stream runs at 2.4 GHz from the start. The dummies write
        # the prefix PSUM tile, which the real chain clears via start=True.
        scr = consts.tile([128, C], BF16, name="warm_scr")
        nc.vector.memset(scr[:], 0.0)
        pkv1 = psm2.tile([2 * K, PRE], F32, tag="p2", name="pkv1")
        for _ in range(50):
            nc.tensor.matmul(pkv1[:], scr[:], scr[:], start=True, stop=True)

        # ============ prefix projection + state (1 chunk) ============
        for ec in range(NEC):
            nc.tensor.matmul(pkv1[:], wkv[:, ts(ec, 2 * K)], xp[:, ts(ec, PRE)],
                             start=(ec == 0), stop=(ec == NEC - 1))
        kT1 = spool.tile([K, PRE], BF16, tag="kT1", name="kT1")
        nc.scalar.activation(kT1[:], pkv1[0:K, :], SIG, bias=bk_ap)
        vT1 = spool.tile([2 * K, PRE], BF16, tag="vT1", name="vT1")
        nc.vector.tensor_copy(vT1[K : 2 * K, :], pkv1[K : 2 * K, :])
        if has_bv:
            nc.vector.tensor_scalar_add(vT1[K : 2 * K, :], vT1[K : 2 * K, :], bv_ap)

        kn1, v1_b, veo1 = transposes(kT1, vT1, 1, "1")
        pu1 = psm2.tile([K, 2 * V], F32, tag="p2", name="pu1")
        nc.tensor.matmul(pu1[:, 0:V], kn1[:], veo1[:, 0:V], start=True, stop=True)
        nc.tensor.matmul(pu1[:, V : 2 * V], kn1[:], veo1[:, V : 2 * V],
                         start=True, stop=True)
        nc.vector.tensor_scalar_mul(geo_bf[:, ts2(0)], pu1[:],
                                    gamma[0:K, G_GAMMA : G_GAMMA + 1])

        # ============ group projections (PE chases the x stream) ============
        vT_sbs = []

        def proj_group(g):
            pkv = pmain.tile([2 * K, GW], F32, tag="pM", name=f"pkv_{g}")
            pqg = pmain.tile([K, GW], F32, tag="pM", name=f"pq_{g}")
            for hf in range(2):
                xh = xg2s[g][:, hf * HCOL : (hf + 1) * HCOL]
                dst = slice(hf * HW2, hf * HW2 + HW2)
                for ec in range(NEC):
                    nc.tensor.matmul(pkv[0 : 2 * K, dst], wkv[:, ts(ec, 2 * K)],
                                     xh[:, ts(ec, HW2)],
                                     start=(ec == 0), stop=(ec == NEC - 1))
                for ec in range(NEC):
                    nc.tensor.matmul(pqg[0:K, dst], wq[:, ts(ec, K)],
                                     xh[:, ts(ec, HW2)],
                                     start=(ec == 0), stop=(ec == NEC - 1))
            nc.scalar.activation(kT_all[:, ts(g, GW)], pkv[0:K, :], SIG, bias=bk_ap)
            vT_sb = spool.tile([2 * K, GW], BF16, tag=f"vT{g}", name=f"vT_sb{g}")
            nc.vector.tensor_copy(vT_sb[K : 2 * K, :], pkv[K : 2 * K, :])
            if has_bv:
                nc.vector.tensor_scalar_add(
                    vT_sb[K : 2 * K, :], vT_sb[K : 2 * K, :], bv_ap)
            vT_sbs.append(vT_sb)
            nc.scalar.activation(qT_all[:, ts(g, GW)], pqg[:], SIG, bias=bq_ap)

        proj_group(0)

        pu2 = pstate.tile([K, (NCH - 1) * 2 * V], F32, name="pu2")

        # stage2 g0: transposes, scores, state updates, chain 1-3
        kn_0, v_0, veo_0 = transposes(kT_all[:, 0:GW], vT_sbs[0], GCH, "20")
        sT_0, qTe_0, qTo_0 = scores_stage(0)
        state_stage(0, kn_0, veo_0)
        for i in range(1, GCH):
            chain_step(i)

        proj_group(1)

        # stage2 g1 part A: transposes + scores (ring-ordered after proj g1)
        kn_1, v_1, veo_1 = transposes(kT_all[:, GW : 2 * GW], vT_sbs[1], GCH, "21")
        sT_1, qTe_1, qTo_1 = scores_stage(1)
        state_stage(1, kn_1, veo_1)

        # attn + out, group 0 (chunk pairs)
        for cl in range(0, GCH, 2):
            attn_out_pair(0, cl, v_0, sT_0, qTe_0, qTo_0)

        # chain into group 1, attn + out group 1
        for cl in range(0, GCH, 2):
            chain_step(GCH + cl)
            chain_step(GCH + cl + 1)
            attn_out_pair(1, cl, v_1, sT_1, qTe_1, qTo_1)

    nc.compile()
    return nc


_CACHE = {}


def _get_program(has_bv):
    key = ("nc", has_bv)
    if key not in _CACHE:
        _CACHE[key] = _build_program(has_bv)
    return _CACHE[key]


def _make_in_maps(x, Wk, bk, Wv, bv, Wq, bq, Wo):
    import ml_dtypes

    bfd = ml_dtypes.bfloat16
    consts = _host_constants()

    def pack2(Wa, Wb):
        # [128, NEC*(outA+outB)]: per embed sub-chunk, [Wa_ec | Wb_ec] columns
        Wab = np.concatenate(
            [Wa.T.reshape(NEC, 128, -1), Wb.T.reshape(NEC, 128, -1)], 2
        )
        return np.ascontiguousarray(
            Wab.transpose(1, 0, 2).reshape(128, -1)
        )

    def pack1(W):
        return np.ascontiguousarray(
            W.T.reshape(NEC, 128, -1).transpose(1, 0, 2).reshape(128, -1)
        )

    identhi = np.zeros((128, 64), np.float32)
    identhi[64:128, :] = np.eye(64)

    def setreg(pack, layout, name, arr):
        r, o, c = layout[name]
        pack[0:r, o : o + c] = arr

    cwb = np.zeros((K, CWB_W), np.float32)
    setreg(cwb, _CWB, "cemat", consts["cemat"])
    setreg(cwb, _CWB, "comat", consts["comat"])

    gcol = np.zeros((128, G_NCOL), np.float32)
    gcol[0:K, G_BKV] = bk
    gcol[K : 2 * K, G_BKV] = bv
    gcol[0:K, G_BQ] = bq
    gcol[:, G_WGE] = consts["wge"]
    gcol[:, G_WGO] = consts["wgo"]

    hdr = np.zeros((128, HDR_W), np.float32)
    setreg(hdr, _HDR, "wkv", pack2(Wk, Wv))
    setreg(hdr, _HDR, "wq", pack1(Wq))
    setreg(hdr, _HDR, "ident", consts["ident64"])
    setreg(hdr, _HDR, "identhi", identhi)
    wrep = np.concatenate([
        np.tile(consts["wge"][:, None], (1, GCH * V)),
        np.tile(consts["wgo"][:, None], (1, GCH * V)),
    ], 1)
    setreg(hdr, _HDR, "wrep", wrep)

    shared = {
        "mloc": consts["mlocT4"].astype(bfd),
        "cwb": cwb.astype(bfd),
        "wo2": np.ascontiguousarray(
            np.concatenate([Wo.T, Wo.T], 0)).astype(bfd),
    }

    def pack_x(xh):
        # [E, HT] -> [NG*2, 128, (ec, t)]: one contiguous block per (g, half)
        v = xh.reshape(NEC, 128, NG, 2, HW2).transpose(2, 3, 1, 0, 4)
        return np.ascontiguousarray(v.reshape(NG * 2, 128, HCOL)).astype(bfd)

    def pack_pre(xh):
        # last PRE prefix rows -> [128, (ec, t)] contiguous block
        v = xh[:, HT - PRE :].reshape(NEC, 128, PRE).transpose(1, 0, 2)
        return np.ascontiguousarray(v.reshape(128, NEC * PRE))

    zeros_pre = np.zeros((128, NEC * PRE), bfd)
    in_maps = []
    for c in range(N_CORES):
        b, h = c // 2, c % 2
        xbT = np.ascontiguousarray(x[b].T)  # [E, T]
        m = dict(shared)
        hh = hdr.copy()
        g = gcol.copy()
        g[:, G_GAMMA] = float(h)
        setreg(hh, _HDR, "gamma", g)
        m["hdr"] = hh.astype(bfd)
        m["xpre"] = (pack_pre(xbT[:, :HT]).astype(bfd) if h == 1 else zeros_pre)
        m["xq4"] = pack_x(xbT[:, h * HT : (h + 1) * HT])
        in_maps.append(m)
    return in_maps


def run(inputs, trace=False):
    """Run on 8 cores; returns (output, BassKernelResults)."""
    inp = {k: np.asarray(v) for k, v in inputs.items()}
    has_bv = bool(np.any(inp["bv"]))
    nc = _get_program(has_bv)
    in_maps = _make_in_maps(**inp)
    res = bass_utils.run_bass_kernel_spmd(
        nc, in_maps, core_ids=list(range(N_CORES)), trace=trace
    )
    out = np.empty((B, T, E), np.float32)
    for c in range(N_CORES):
        b, h = c // 2, c % 2
        o = res.results[c]["out"].astype(np.float32)  # [NCH, 128, E]
        out[b, h * HT : (h + 1) * HT, :] = o.reshape(HT, E)
    return out, res


def kernel(**inputs):
    out, _ = run(inputs, trace=False)
    return out


# revision 51
# speedup vs baseline: 1.0261x; 1.0261x over previous
"""Trainium2 Bass kernel for nn_Decay2D (decay-masked linear attention).

Math: the reference's Hillis-Steele scan with decay-squaring order composes
to coefficient d^ceil((t-s)/2) on store[s] = scale*k_s v_s^T, so

    out[t] = scale^2 * sum_{s<=t} d^ceil((t-s)/2) (q_t . k_s) v_s  @ Wo^T

computed as chunked linear attention with a [K, 2V] carry state per chunk
(even/odd decay chains on the V axis), never materializing [B,T,K,V].

Sharding: 8 cores = 4 batches x 2 sequence halves. Each core builds the
carry state over a truncated 128-row prefix and runs full attention +
output projection for its own 1024 rows.

v11 (~44.5us vs 50.2us v3 baseline): the body is built around the DMA
stream. All transfers are contiguous DRAM blocks (header with
gamma/weights/idents/wrep, prefix block, per-(group,half) x blocks, mloc,
a 64-row ce/co/Wo block, per-chunk output stores) on the sync HWDGE ring
in consumption order; the PE chases the stream. ~50 warm-up matmuls on a
memset scratch run while the header streams in so HAM reaches K=8/8
before real work. PSUM rings are split so the attention accumulators
(plt) ping-pong independently of the projection/score/out-proj ring. The
carry chain is bf16 end-to-end (one DVE stt per chunk, no casts), and the
parity weights ride a header-shipped broadcast block (wrep) because both
DVE and POOL tensor_scalar are slow paths (~4x and ~13x vs tensor_mul).
"""

from contextlib import ExitStack

import numpy as np

import concourse.bass as bass
import concourse.bacc as bacc
import concourse.mybir as mybir
import concourse.tile as tile
from concourse import bass_utils
from concourse.alu_op_type import AluOpType
from concourse.bass import ts

F32 = mybir.dt.float32
BF16 = mybir.dt.bfloat16
SIG = mybir.ActivationFunctionType.Sigmoid

B, T, E, K, V = 4, 2048, 1024, 64, 64
DECAY = 0.9
C = 128          # chunk length
HT = T // 2      # rows per core (sequence half)
NCH = HT // C    # chunks per half (8)
NEC = E // 128   # embed sub-chunks (8)
GW = 512         # group width: 4 chunks per PSUM bank
GCH = GW // C    # chunks per group (4)
NG = HT // GW    # groups per half (2)
HW2 = GW // 2    # half-group width (256)
HCOL = NEC * HW2 # x columns per (group, half) block (2048)
DC2 = float(DECAY ** (C // 2))
N_CORES = 8
PRE = 128        # truncated prefix length (1 chunk; older rows decay < 2e-3)

def _mklayout(regions):
    out, off = {}, 0
    for n, r, c in regions:
        out[n] = (r, off, c)
        off += c
    return out, off


# gamma columns (bf16, inside hdr): 0 prefix flag, 1 bk|bv, 2 bq, 3 wge, 4 wgo
G_GAMMA, G_BKV, G_BQ, G_WGE, G_WGO = 0, 1, 2, 3, 4
G_NCOL = 8

_HDR, HDR_W = _mklayout([
    ("gamma", 128, G_NCOL),
    ("wkv", 128, NEC * 2 * K), ("wq", 128, NEC * K),
    ("ident", 64, 64), ("identhi", 128, 64),
    ("wrep", 128, 2 * GCH * V),
])
_CWB, CWB_W = _mklayout([
    ("cemat", K, GW), ("comat", K, GW),
])


def _host_constants():
    d = DECAY
    scale2 = 1.0 - d
    i = np.arange(C)
    j = np.arange(C)
    delta = i[:, None] - j[None, :]
    # intra-chunk decay mask, transposed to [tcol(j), trow(i)], scale^2 folded
    mloc = np.where(delta >= 0, d ** np.ceil(delta / 2.0), 0.0) * scale2
    mlocT4 = np.tile(np.ascontiguousarray(mloc.T), (1, GCH)).astype(np.float32)
    # boundary coefficient per local row i (scale^2 folded), split by parity
    c = d ** np.ceil((i + 1) / 2.0) * scale2
    ce = np.where(i % 2 == 0, c, 0.0).astype(np.float32)
    co = np.where(i % 2 == 1, c, 0.0).astype(np.float32)
    cemat = np.tile(np.broadcast_to(ce, (K, C)), (1, GCH)).astype(np.float32)
    comat = np.tile(np.broadcast_to(co, (K, C)), (1, GCH)).astype(np.float32)
    # state-update row weights (per t within chunk)
    u_o = np.where(j % 2 == 1, d ** ((C - 1 - j) / 2.0), 0.0)
    u_e = np.where(j % 2 == 0, d ** ((C - 2 - j) / 2.0), 0.0)
    wge = (u_o + u_e).astype(np.float32)          # [C]
    wgo = (u_o + d * u_e).astype(np.float32)
    return {
        "mlocT4": mlocT4,
        "cemat": np.ascontiguousarray(cemat),
        "comat": np.ascontiguousarray(comat),
        "wge": wge,
        "wgo": wgo,
        "ident64": np.eye(64, dtype=np.float32),
    }


def _build_program(has_bv):
    nc = bacc.Bacc(
        "TRN2",
        debug=False,
        enable_asserts=False,
        target_bir_lowering=False,
        num_devices=N_CORES,
    )

    def din(name, shape, dtype=BF16):
        return nc.dram_tensor(name, shape, dtype, kind="ExternalInput").ap()

    hdr_d = din("hdr", [128, HDR_W])
    xpre_d = din("xpre", [128, NEC * PRE])        # prefix x, (ec, t)
    xq4 = din("xq4", [NG * 2, 128, HCOL])         # x per (group, half), (ec, t)
    mloc_d = din("mloc", [C, GW])
    cwb_d = din("cwb", [K, CWB_W])                # ce / co (64 rows)
    wo2_d = din("wo2", [128, E])                  # Wo^T duplicated in both halves
    out_d = nc.dram_tensor("out", [NCH, 128, E], BF16,
                           kind="ExternalOutput").ap()

    with ExitStack() as ctx:
        tc = ctx.enter_context(tile.TileContext(nc))

        consts = ctx.enter_context(tc.tile_pool(name="consts", bufs=1))
        state = ctx.enter_context(tc.tile_pool(name="state", bufs=1))
        xpool = ctx.enter_context(tc.tile_pool(name="xg", bufs=2))
        spool = ctx.enter_context(tc.tile_pool(name="sml", bufs=2))
        opool = ctx.enter_context(tc.tile_pool(name="osb", bufs=3))
        # PSUM budget (8 banks): pmain ring 2 (pkv/pq/ps/po), psm2 ring 2
        # (pkv1/pu1/plt ping-pong), psml ring 2 (transposes), pstate 2 (pu2)
        pmain = ctx.enter_context(tc.tile_pool(name="pmain", bufs=2, space="PSUM"))
        psm2 = ctx.enter_context(tc.tile_pool(name="psm2", bufs=2, space="PSUM"))
        psml = ctx.enter_context(tc.tile_pool(name="psml", bufs=2, space="PSUM"))
        pstate = ctx.enter_context(tc.tile_pool(name="pstate", bufs=1, space="PSUM"))

        # ---- loads in wire order (single sync HWDGE ring = FIFO) ----
        hdr = consts.tile([128, HDR_W], BF16, name="hdr")
        nc.sync.dma_start(hdr[:], hdr_d[:])
        xp = consts.tile([128, NEC * PRE], BF16, name="xp")
        nc.sync.dma_start(xp[:], xpre_d[:])

        xg2s = [xpool.tile([128, 2 * HCOL], BF16, tag="xg", name=f"xg2_{g}")
                for g in range(NG)]

        def load_xhalf(g, hf):
            nc.sync.dma_start(
                xg2s[g][:, hf * HCOL : (hf + 1) * HCOL], xq4[g * 2 + hf])

        load_xhalf(0, 0)
        load_xhalf(0, 1)
        load_xhalf(1, 0)
        load_xhalf(1, 1)
        mlocT4 = consts.tile([C, GW], BF16, name="mloc")
        nc.sync.dma_start(mlocT4[:], mloc_d[:])
        cwb = consts.tile([K, CWB_W], BF16, name="cwb")
        nc.sync.dma_start(cwb[:], cwb_d[:])
        wo2 = consts.tile([128, E], BF16, name="wo2")
        nc.sync.dma_start(wo2[:], wo2_d[:])

        def reg(pack, layout, name):
            r, o, c = layout[name]
            return pack[0:r, o : o + c]

        gamma = consts.tile([128, G_NCOL], F32, name="gamma_f32")
        nc.vector.tensor_copy(gamma[:], reg(hdr, _HDR, "gamma"))
        wkv, wq = reg(hdr, _HDR, "wkv"), reg(hdr, _HDR, "wq")
        ident, identhi = reg(hdr, _HDR, "ident"), reg(hdr, _HDR, "identhi")
        cemat, comat = reg(cwb, _CWB, "cemat"), reg(cwb, _CWB, "comat")
        bk_ap = gamma[0:K, G_BKV : G_BKV + 1]
        bv_ap = gamma[K : 2 * K, G_BKV : G_BKV + 1]
        bq_ap = gamma[0:K, G_BQ : G_BQ + 1]
        wge_ap = gamma[:, G_WGE : G_WGE + 1]
        wgo_ap = gamma[:, G_WGO : G_WGO + 1]

        qT_all = consts.tile([K, HT], BF16, name="qT_all")
        kT_all = consts.tile([K, HT], BF16, name="kT_all")
        lt_all = consts.tile([V, HT], BF16, name="lt_all")
        # carry state: parity on the V axis -> [K, NCH * 2V], kept in bf16
        # (the 7-step chain loses ~0.3% which is well inside tolerance)
        geo_bf = state.tile([K, NCH * 2 * V], BF16, name="geo_bf")
        # wge/wgo broadcast into [C, GCH*V] blocks, shipped in the header
        wrep = reg(hdr, _HDR, "wrep")

        def ts2(i):  # [K, 2V] slice of the state for chunk i
            return slice(i * 2 * V, (i + 1) * 2 * V)

        # ---- stage helpers ----
        def transposes(kT_src, vT_src, nch, tagp):
            pkn = psml.tile([C, nch * K], BF16, tag="pS", name=f"pkn{tagp}")
            for cl in range(nch):
                nc.tensor.matmul(pkn[:, ts(cl, K)], kT_src[:, ts(cl, C)],
                                 ident[:], is_transpose=True)
            pvn = psml.tile([C, nch * V], BF16, tag="pS", name=f"pvn{tagp}")
            for cl in range(nch):
                nc.tensor.matmul(pvn[:, ts(cl, V)], vT_src[K : 2 * K, ts(cl, C)],
                                 identhi[K : 2 * K, :], is_transpose=True)
            kn = spool.tile([C, nch * K], BF16, tag=f"kn{tagp}", name=f"kn{tagp}")
            nc.scalar.copy(kn[:], pkn[:])
            v_b = spool.tile([C, nch * V], BF16, tag=f"v{tagp}", name=f"v{tagp}")
            nc.vector.tensor_copy(v_b[:], pvn[:])
            # parity-weighted v for the state update
            veo = spool.tile([C, 2 * nch * V], BF16, tag=f"veo{tagp}",
                             name=f"veo{tagp}")
            nc.vector.tensor_mul(veo[:, 0 : nch * V], v_b[:],
                                 wrep[:, 0 : nch * V])
            nc.vector.tensor_mul(veo[:, nch * V :], v_b[:],
                                 wrep[:, GCH * V : (GCH + nch) * V])
            return kn, v_b, veo

        def scores_stage(g):
            ps = pmain.tile([C, GW], F32, tag="pM", name=f"ps{g}")
            for cl in range(GCH):
                i = g * GCH + cl
                nc.tensor.matmul(ps[:, ts(cl, C)], kT_all[:, ts(i, C)],
                                 qT_all[:, ts(i, C)], start=True, stop=True)
            sT_b = spool.tile([C, GW], BF16, tag=f"sm{g}", name="sT_b")
            nc.vector.tensor_mul(sT_b[:], ps[:], mlocT4[:])
            qTe = spool.tile([K, GW], BF16, tag=f"qe{g}", name="qTe")
            nc.vector.tensor_mul(qTe[:], qT_all[:, ts(g, GW)], cemat[:])
            qTo = spool.tile([K, GW], BF16, tag=f"qo{g}", name="qTo")
            nc.gpsimd.tensor_mul(qTo[:], qT_all[:, ts(g, GW)], comat[:])
            return sT_b, qTe, qTo

        def state_stage(g, kn_g, veo_g):
            # pu2 updates for this group's chunks (skip the last chunk)
            for cl in range(GCH):
                i = g * GCH + cl
                if i < NCH - 1:
                    nc.tensor.matmul(pu2[:, i * 2 * V : i * 2 * V + V],
                                     kn_g[:, ts(cl, K)],
                                     veo_g[:, ts(cl, V)],
                                     start=True, stop=True)
                    nc.tensor.matmul(pu2[:, i * 2 * V + V : (i + 1) * 2 * V],
                                     kn_g[:, ts(cl, K)],
                                     veo_g[:, (GCH + cl) * V : (GCH + cl + 1) * V],
                                     start=True, stop=True)

        def chain_step(i):
            nc.vector.scalar_tensor_tensor(
                geo_bf[:, ts2(i)], geo_bf[:, ts2(i - 1)], DC2,
                pu2[:, ts2(i - 1)], AluOpType.mult, AluOpType.add,
            )

        def attn_out_chunk(g, cl, v_b, sT_b, qTe, qTo):
            i = g * GCH + cl
            plt = psm2.tile([V, C], F32, tag="p2", name=f"plt{i}")
            nc.tensor.matmul(plt[:], v_b[:, ts(cl, V)], sT_b[:, ts(cl, C)],
                             start=True, stop=False)
            nc.tensor.matmul(plt[:], geo_bf[:, i * 2 * V : i * 2 * V + V],
                             qTe[:, ts(cl, C)], start=False, stop=False)
            nc.tensor.matmul(plt[:], geo_bf[:, i * 2 * V + V : (i + 1) * 2 * V],
                             qTo[:, ts(cl, C)], start=False, stop=True)
            if cl % 2 == 0:
                nc.scalar.copy(lt_all[:, ts(i, C)], plt[:])
            else:
                nc.vector.tensor_copy(lt_all[:, ts(i, C)], plt[:])
            out_sb = opool.tile([C, E], BF16, tag="osb", name=f"out_sb{i}")
            for h in range(2):
                po = pmain.tile([C, GW], F32, tag="pM", name=f"po{i}_{h}")
                nc.tensor.matmul(po[:], lt_all[:, ts(i, C)],
                                 wo2[0:64, ts(h, GW)], start=True, stop=True)
                dst = out_sb[:, ts(h, GW)]
                if h == 0:
                    nc.scalar.copy(dst, po[:])
                else:
                    nc.vector.tensor_copy(dst, po[:])
            nc.sync.dma_start(out_d[i], out_sb[:])

        # ============ PE warm-up ============
        # ~35 filler matmuls on a memset scratch while the header streams
        # in: HAM flips to K=8/8 (~3.4us of sustained PE busy) so the real
        # matmul stream runs at 2.4 GHz from the start. The dummies write
        # the prefix PSUM tile, which the real chain clears via start=True.
        scr = consts.tile([128, C], BF16, name="warm_scr")
        nc.vector.memset(scr[:], 0.0)
        pkv1 = psm2.tile([2 * K, PRE], F32, tag="p2", name="pkv1")
        for _ in range(50):
            nc.tensor.matmul(pkv1[:], scr[:], scr[:], start=True, stop=True)

        # ============ prefix projection + state (1 chunk) ============
        for ec in range(NEC):
            nc.tensor.matmul(pkv1[:], wkv[:, ts(ec, 2 * K)], xp[:, ts(ec, PRE)],
                             start=(ec == 0), stop=(ec == NEC - 1))
        kT1 = spool.tile([K, PRE], BF16, tag="kT1", name="kT1")
        nc.scalar.activation(kT1[:], pkv1[0:K, :], SIG, bias=bk_ap)
        vT1 = spool.tile([2 * K, PRE], BF16, tag="vT1", name="vT1")
        nc.vector.tensor_copy(vT1[K : 2 * K, :], pkv1[K : 2 * K, :])
        if has_bv:
            nc.vector.tensor_scalar_add(vT1[K : 2 * K, :], vT1[K : 2 * K, :], bv_ap)

        kn1, v1_b, veo1 = transposes(kT1, vT1, 1, "1")
        pu1 = psm2.tile([K, 2 * V], F32, tag="p2", name="pu1")
        nc.tensor.matmul(pu1[:, 0:V], kn1[:], veo1[:, 0:V], start=True, stop=True)
        nc.tensor.matmul(pu1[:, V : 2 * V], kn1[:], veo1[:, V : 2 * V],
                         start=True, stop=True)
        nc.vector.tensor_scalar_mul(geo_bf[:, ts2(0)], pu1[:],
                                    gamma[0:K, G_GAMMA : G_GAMMA + 1])

        # ============ group projections (PE chases the x stream) ============
        vT_sbs = []

        def proj_group(g):
            pkv = pmain.tile([2 * K, GW], F32, tag="pM", name=f"pkv_{g}")
            pqg = pmain.tile([K, GW], F32, tag="pM", name=f"pq_{g}")
            for hf in range(2):
                xh = xg2s[g][:, hf * HCOL : (hf + 1) * HCOL]
                dst = slice(hf * HW2, hf * HW2 + HW2)
                for ec in range(NEC):
                    nc.tensor.matmul(pkv[0 : 2 * K, dst], wkv[:, ts(ec, 2 * K)],
                                     xh[:, ts(ec, HW2)],
                                     start=(ec == 0), stop=(ec == NEC - 1))
                for ec in range(NEC):
                    nc.tensor.matmul(pqg[0:K, dst], wq[:, ts(ec, K)],
                                     xh[:, ts(ec, HW2)],
                                     start=(ec == 0), stop=(ec == NEC - 1))
            nc.scalar.activation(kT_all[:, ts(g, GW)], pkv[0:K, :], SIG, bias=bk_ap)
            vT_sb = spool.tile([2 * K, GW], BF16, tag=f"vT{g}", name=f"vT_sb{g}")
            nc.vector.tensor_copy(vT_sb[K : 2 * K, :], pkv[K : 2 * K, :])
            if has_bv:
                nc.vector.tensor_scalar_add(
                    vT_sb[K : 2 * K, :], vT_sb[K : 2 * K, :], bv_ap)
            vT_sbs.append(vT_sb)
            nc.scalar.activation(qT_all[:, ts(g, GW)], pqg[:], SIG, bias=bq_ap)

        proj_group(0)

        pu2 = pstate.tile([K, (NCH - 1) * 2 * V], F32, name="pu2")

        # stage2 g0: transposes, scores, state updates, chain 1-3
        kn_0, v_0, veo_0 = transposes(kT_all[:, 0:GW], vT_sbs[0], GCH, "20")
        sT_0, qTe_0, qTo_0 = scores_stage(0)
        state_stage(0, kn_0, veo_0)
        for i in range(1, GCH):
            chain_step(i)

        proj_group(1)

        # stage2 g1 part A: transposes + scores (ring-ordered after proj g1)
        kn_1, v_1, veo_1 = transposes(kT_all[:, GW : 2 * GW], vT_sbs[1], GCH, "21")
        sT_1, qTe_1, qTo_1 = scores_stage(1)
        state_stage(1, kn_1, veo_1)

        # attn + out, group 0
        for cl in range(GCH):
            attn_out_chunk(0, cl, v_0, sT_0, qTe_0, qTo_0)

        # chain into group 1, attn + out group 1
        for cl in range(GCH):
            chain_step(GCH + cl)
            attn_out_chunk(1, cl, v_1, sT_1, qTe_1, qTo_1)

    nc.compile()
    return nc


_CACHE = {}


def _get_program(has_bv):
    key = ("nc", has_bv)
    if key not in _CACHE:
        _CACHE[key] = _build_program(has_bv)
    return _CACHE[key]


def _make_in_maps(x, Wk, bk, Wv, bv, Wq, bq, Wo):
    import ml_dtypes

    bfd = ml_dtypes.bfloat16
    consts = _host_constants()

    def pack2(Wa, Wb):
        # [128, NEC*(outA+outB)]: per embed sub-chunk, [Wa_ec | Wb_ec] columns
        Wab = np.concatenate(
            [Wa.T.reshape(NEC, 128, -1), Wb.T.reshape(NEC, 128, -1)], 2
        )
        return np.ascontiguousarray(
            Wab.transpose(1, 0, 2).reshape(128, -1)
        )

    def pack1(W):
        return np.ascontiguousarray(
            W.T.reshape(NEC, 128, -1).transpose(1, 0, 2).reshape(128, -1)
        )

    identhi = np.zeros((128, 64), np.float32)
    identhi[64:128, :] = np.eye(64)

    def setreg(pack, layout, name, arr):
        r, o, c = layout[name]
        pack[0:r, o : o + c] = arr

    cwb = np.zeros((K, CWB_W), np.float32)
    setreg(cwb, _CWB, "cemat", consts["cemat"])
    setreg(cwb, _CWB, "comat", consts["comat"])

    gcol = np.zeros((128, G_NCOL), np.float32)
    gcol[0:K, G_BKV] = bk
    gcol[K : 2 * K, G_BKV] = bv
    gcol[0:K, G_BQ] = bq
    gcol[:, G_WGE] = consts["wge"]
    gcol[:, G_WGO] = consts["wgo"]

    hdr = np.zeros((128, HDR_W), np.float32)
    setreg(hdr, _HDR, "wkv", pack2(Wk, Wv))
    setreg(hdr, _HDR, "wq", pack1(Wq))
    setreg(hdr, _HDR, "ident", consts["ident64"])
    setreg(hdr, _HDR, "identhi", identhi)
    wrep = np.concatenate([
        np.tile(consts["wge"][:, None], (1, GCH * V)),
        np.tile(consts["wgo"][:, None], (1, GCH * V)),
    ], 1)
    setreg(hdr, _HDR, "wrep", wrep)

    shared = {
        "mloc": consts["mlocT4"].astype(bfd),
        "cwb": cwb.astype(bfd),
        "wo2": np.ascontiguousarray(
            np.concatenate([Wo.T, Wo.T], 0)).astype(bfd),
    }

    def pack_x(xh):
        # [E, HT] -> [NG*2, 128, (ec, t)]: one contiguous block per (g, half)
        v = xh.reshape(NEC, 128, NG, 2, HW2).transpose(2, 3, 1, 0, 4)
        return np.ascontiguousarray(v.reshape(NG * 2, 128, HCOL)).astype(bfd)

    def pack_pre(xh):
        # last PRE prefix rows -> [128, (ec, t)] contiguous block
        v = xh[:, HT - PRE :].reshape(NEC, 128, PRE).transpose(1, 0, 2)
        return np.ascontiguousarray(v.reshape(128, NEC * PRE))

    zeros_pre = np.zeros((128, NEC * PRE), bfd)
    in_maps = []
    for c in range(N_CORES):
        b, h = c // 2, c % 2
        xbT = np.ascontiguousarray(x[b].T)  # [E, T]
        m = dict(shared)
        hh = hdr.copy()
        g = gcol.copy()
        g[:, G_GAMMA] = float(h)
        setreg(hh, _HDR, "gamma", g)
        m["hdr"] = hh.astype(bfd)
        m["xpre"] = (pack_pre(xbT[:, :HT]).astype(bfd) if h == 1 else zeros_pre)
        m["xq4"] = pack_x(xbT[:, h * HT : (h + 1) * HT])
        in_maps.append(m)
    return in_maps


def run(inputs, trace=False):
    """Run on 8 cores; returns (output, BassKernelResults)."""
    inp = {k: np.asarray(v) for k, v in inputs.items()}
    has_bv = bool(np.any(inp["bv"]))
    nc = _get_program(has_bv)
    in_maps = _make_in_maps(**inp)
    res = bass_utils.run_bass_kernel_spmd(
        nc, in_maps, core_ids=list(range(N_CORES)), trace=trace
    )
    out = np.empty((B, T, E), np.float32)
    for c in range(N_CORES):
        b, h = c // 2, c % 2
        o = res.results[c]["out"].astype(np.float32)  # [NCH, 128, E]
        out[b, h * HT : (h + 1) * HT, :] = o.reshape(HT, E)
    return out, res


def kernel(**inputs):
    out, _ = run(inputs, trace=False)
    return out


# revision 52
# speedup vs baseline: 1.0303x; 1.0041x over previous
"""Trainium2 Bass kernel for nn_Decay2D (decay-masked linear attention).

Math: the reference's Hillis-Steele scan with decay-squaring order composes
to coefficient d^ceil((t-s)/2) on store[s] = scale*k_s v_s^T, so

    out[t] = scale^2 * sum_{s<=t} d^ceil((t-s)/2) (q_t . k_s) v_s  @ Wo^T

computed as chunked linear attention with a [K, 2V] carry state per chunk
(even/odd decay chains on the V axis), never materializing [B,T,K,V].

Sharding: 8 cores = 4 batches x 2 sequence halves. Each core builds the
carry state over a truncated 128-row prefix and runs full attention +
output projection for its own 1024 rows.

v11 (~44.5us vs 50.2us v3 baseline): the body is built around the DMA
stream. All transfers are contiguous DRAM blocks (header with
gamma/weights/idents/wrep, prefix block, per-(group,half) x blocks, mloc,
a 64-row ce/co/Wo block, per-chunk output stores) on the sync HWDGE ring
in consumption order; the PE chases the stream. ~50 warm-up matmuls on a
memset scratch run while the header streams in so HAM reaches K=8/8
before real work. PSUM rings are split so the attention accumulators
(plt) ping-pong independently of the projection/score/out-proj ring. The
carry chain is bf16 end-to-end (one DVE stt per chunk, no casts), and the
parity weights ride a header-shipped broadcast block (wrep) because both
DVE and POOL tensor_scalar are slow paths (~4x and ~13x vs tensor_mul).
"""

from contextlib import ExitStack

import numpy as np

import concourse.bass as bass
import concourse.bacc as bacc
import concourse.mybir as mybir
import concourse.tile as tile
from concourse import bass_utils
from concourse.alu_op_type import AluOpType
from concourse.bass import ts

F32 = mybir.dt.float32
BF16 = mybir.dt.bfloat16
SIG = mybir.ActivationFunctionType.Sigmoid

B, T, E, K, V = 4, 2048, 1024, 64, 64
DECAY = 0.9
C = 128          # chunk length
HT = T // 2      # rows per core (sequence half)
NCH = HT // C    # chunks per half (8)
NEC = E // 128   # embed sub-chunks (8)
GW = 512         # group width: 4 chunks per PSUM bank
GCH = GW // C    # chunks per group (4)
NG = HT // GW    # groups per half (2)
HW2 = GW // 2    # half-group width (256)
HCOL = NEC * HW2 # x columns per (group, half) block (2048)
DC2 = float(DECAY ** (C // 2))
N_CORES = 8
PRE = 128        # truncated prefix length (1 chunk; older rows decay < 2e-3)

def _mklayout(regions):
    out, off = {}, 0
    for n, r, c in regions:
        out[n] = (r, off, c)
        off += c
    return out, off


# gamma columns (bf16, inside hdr): 0 prefix flag, 1 bk|bv, 2 bq, 3 wge, 4 wgo
G_GAMMA, G_BKV, G_BQ, G_WGE, G_WGO = 0, 1, 2, 3, 4
G_NCOL = 8

_HDR, HDR_W = _mklayout([
    ("gamma", 128, G_NCOL),
    ("wkv", 128, NEC * 2 * K), ("wq", 128, NEC * K),
    ("ident", 64, 64), ("identhi", 128, 64),
    ("wrep", 128, 2 * GCH * V),
])
_CWB, CWB_W = _mklayout([
    ("cemat", K, GW), ("comat", K, GW),
])


def _host_constants():
    d = DECAY
    scale2 = 1.0 - d
    i = np.arange(C)
    j = np.arange(C)
    delta = i[:, None] - j[None, :]
    # intra-chunk decay mask, transposed to [tcol(j), trow(i)], scale^2 folded
    mloc = np.where(delta >= 0, d ** np.ceil(delta / 2.0), 0.0) * scale2
    mlocT4 = np.tile(np.ascontiguousarray(mloc.T), (1, GCH)).astype(np.float32)
    # boundary coefficient per local row i (scale^2 folded), split by parity
    c = d ** np.ceil((i + 1) / 2.0) * scale2
    ce = np.where(i % 2 == 0, c, 0.0).astype(np.float32)
    co = np.where(i % 2 == 1, c, 0.0).astype(np.float32)
    cemat = np.tile(np.broadcast_to(ce, (K, C)), (1, GCH)).astype(np.float32)
    comat = np.tile(np.broadcast_to(co, (K, C)), (1, GCH)).astype(np.float32)
    # state-update row weights (per t within chunk)
    u_o = np.where(j % 2 == 1, d ** ((C - 1 - j) / 2.0), 0.0)
    u_e = np.where(j % 2 == 0, d ** ((C - 2 - j) / 2.0), 0.0)
    wge = (u_o + u_e).astype(np.float32)          # [C]
    wgo = (u_o + d * u_e).astype(np.float32)
    return {
        "mlocT4": mlocT4,
        "cemat": np.ascontiguousarray(cemat),
        "comat": np.ascontiguousarray(comat),
        "wge": wge,
        "wgo": wgo,
        "ident64": np.eye(64, dtype=np.float32),
    }


def _build_program(has_bv):
    nc = bacc.Bacc(
        "TRN2",
        debug=False,
        enable_asserts=False,
        target_bir_lowering=False,
        num_devices=N_CORES,
    )

    def din(name, shape, dtype=BF16):
        return nc.dram_tensor(name, shape, dtype, kind="ExternalInput").ap()

    hdr_d = din("hdr", [128, HDR_W])
    xpre_d = din("xpre", [128, NEC * PRE])        # prefix x, (ec, t)
    xq4 = din("xq4", [NG * 2, 128, HCOL])         # x per (group, half), (ec, t)
    mloc_d = din("mloc", [C, GW])
    cwb_d = din("cwb", [K, CWB_W])                # ce / co (64 rows)
    wo2_d = din("wo2", [K, E])                    # Wo^T
    out_d = nc.dram_tensor("out", [NCH, 128, E], BF16,
                           kind="ExternalOutput").ap()

    with ExitStack() as ctx:
        tc = ctx.enter_context(tile.TileContext(nc))

        consts = ctx.enter_context(tc.tile_pool(name="consts", bufs=1))
        state = ctx.enter_context(tc.tile_pool(name="state", bufs=1))
        xpool = ctx.enter_context(tc.tile_pool(name="xg", bufs=2))
        spool = ctx.enter_context(tc.tile_pool(name="sml", bufs=2))
        opool = ctx.enter_context(tc.tile_pool(name="osb", bufs=3))
        # PSUM budget (8 banks): pmain ring 2 (pkv/pq/ps/po), psm2 ring 2
        # (pkv1/pu1/plt ping-pong), psml ring 2 (transposes), pstate 2 (pu2)
        pmain = ctx.enter_context(tc.tile_pool(name="pmain", bufs=2, space="PSUM"))
        psm2 = ctx.enter_context(tc.tile_pool(name="psm2", bufs=2, space="PSUM"))
        psml = ctx.enter_context(tc.tile_pool(name="psml", bufs=2, space="PSUM"))
        pstate = ctx.enter_context(tc.tile_pool(name="pstate", bufs=1, space="PSUM"))

        # ---- loads in wire order (single sync HWDGE ring = FIFO) ----
        hdr = consts.tile([128, HDR_W], BF16, name="hdr")
        nc.sync.dma_start(hdr[:], hdr_d[:])
        xp = consts.tile([128, NEC * PRE], BF16, name="xp")
        nc.sync.dma_start(xp[:], xpre_d[:])

        xg2s = [xpool.tile([128, 2 * HCOL], BF16, tag="xg", name=f"xg2_{g}")
                for g in range(NG)]

        def load_xhalf(g, hf):
            nc.sync.dma_start(
                xg2s[g][:, hf * HCOL : (hf + 1) * HCOL], xq4[g * 2 + hf])

        load_xhalf(0, 0)
        load_xhalf(0, 1)
        load_xhalf(1, 0)
        load_xhalf(1, 1)
        mlocT4 = consts.tile([C, GW], BF16, name="mloc")
        nc.sync.dma_start(mlocT4[:], mloc_d[:])
        cwb = consts.tile([K, CWB_W], BF16, name="cwb")
        nc.sync.dma_start(cwb[:], cwb_d[:])
        wo2 = consts.tile([K, E], BF16, name="wo2")
        nc.sync.dma_start(wo2[:], wo2_d[:])

        def reg(pack, layout, name):
            r, o, c = layout[name]
            return pack[0:r, o : o + c]

        gamma = consts.tile([128, G_NCOL], F32, name="gamma_f32")
        nc.vector.tensor_copy(gamma[:], reg(hdr, _HDR, "gamma"))
        wkv, wq = reg(hdr, _HDR, "wkv"), reg(hdr, _HDR, "wq")
        ident, identhi = reg(hdr, _HDR, "ident"), reg(hdr, _HDR, "identhi")
        cemat, comat = reg(cwb, _CWB, "cemat"), reg(cwb, _CWB, "comat")
        bk_ap = gamma[0:K, G_BKV : G_BKV + 1]
        bv_ap = gamma[K : 2 * K, G_BKV : G_BKV + 1]
        bq_ap = gamma[0:K, G_BQ : G_BQ + 1]
        wge_ap = gamma[:, G_WGE : G_WGE + 1]
        wgo_ap = gamma[:, G_WGO : G_WGO + 1]

        qT_all = consts.tile([K, HT], BF16, name="qT_all")
        kT_all = consts.tile([K, HT], BF16, name="kT_all")
        lt_all = consts.tile([V, HT], BF16, name="lt_all")
        # carry state: parity on the V axis -> [K, NCH * 2V], kept in bf16
        # (the 7-step chain loses ~0.3% which is well inside tolerance)
        geo_bf = state.tile([K, NCH * 2 * V], BF16, name="geo_bf")
        # wge/wgo broadcast into [C, GCH*V] blocks, shipped in the header
        wrep = reg(hdr, _HDR, "wrep")

        def ts2(i):  # [K, 2V] slice of the state for chunk i
            return slice(i * 2 * V, (i + 1) * 2 * V)

        # ---- stage helpers ----
        def transposes(kT_src, vT_src, nch, tagp):
            pkn = psml.tile([C, nch * K], BF16, tag="pS", name=f"pkn{tagp}")
            for cl in range(nch):
                nc.tensor.matmul(pkn[:, ts(cl, K)], kT_src[:, ts(cl, C)],
                                 ident[:], is_transpose=True)
            pvn = psml.tile([C, nch * V], BF16, tag="pS", name=f"pvn{tagp}")
            for cl in range(nch):
                nc.tensor.matmul(pvn[:, ts(cl, V)], vT_src[K : 2 * K, ts(cl, C)],
                                 identhi[K : 2 * K, :], is_transpose=True)
            kn = spool.tile([C, nch * K], BF16, tag=f"kn{tagp}", name=f"kn{tagp}")
            nc.scalar.copy(kn[:], pkn[:])
            v_b = spool.tile([C, nch * V], BF16, tag=f"v{tagp}", name=f"v{tagp}")
            nc.vector.tensor_copy(v_b[:], pvn[:])
            # parity-weighted v for the state update
            veo = spool.tile([C, 2 * nch * V], BF16, tag=f"veo{tagp}",
                             name=f"veo{tagp}")
            nc.vector.tensor_mul(veo[:, 0 : nch * V], v_b[:],
                                 wrep[:, 0 : nch * V])
            nc.vector.tensor_mul(veo[:, nch * V :], v_b[:],
                                 wrep[:, GCH * V : (GCH + nch) * V])
            return kn, v_b, veo

        def scores_stage(g):
            ps = pmain.tile([C, GW], F32, tag="pM", name=f"ps{g}")
            for cl in range(GCH):
                i = g * GCH + cl
                nc.tensor.matmul(ps[:, ts(cl, C)], kT_all[:, ts(i, C)],
                                 qT_all[:, ts(i, C)], start=True, stop=True)
            sT_b = spool.tile([C, GW], BF16, tag=f"sm{g}", name="sT_b")
            nc.vector.tensor_mul(sT_b[:], ps[:], mlocT4[:])
            qTe = spool.tile([K, GW], BF16, tag=f"qe{g}", name="qTe")
            nc.vector.tensor_mul(qTe[:], qT_all[:, ts(g, GW)], cemat[:])
            qTo = spool.tile([K, GW], BF16, tag=f"qo{g}", name="qTo")
            nc.gpsimd.tensor_mul(qTo[:], qT_all[:, ts(g, GW)], comat[:])
            return sT_b, qTe, qTo

        def state_stage(g, kn_g, veo_g):
            # pu2 updates for this group's chunks (skip the last chunk)
            for cl in range(GCH):
                i = g * GCH + cl
                if i < NCH - 1:
                    nc.tensor.matmul(pu2[:, i * 2 * V : i * 2 * V + V],
                                     kn_g[:, ts(cl, K)],
                                     veo_g[:, ts(cl, V)],
                                     start=True, stop=True)
                    nc.tensor.matmul(pu2[:, i * 2 * V + V : (i + 1) * 2 * V],
                                     kn_g[:, ts(cl, K)],
                                     veo_g[:, (GCH + cl) * V : (GCH + cl + 1) * V],
                                     start=True, stop=True)

        def chain_step(i):
            nc.vector.scalar_tensor_tensor(
                geo_bf[:, ts2(i)], geo_bf[:, ts2(i - 1)], DC2,
                pu2[:, ts2(i - 1)], AluOpType.mult, AluOpType.add,
            )

        def attn_out_chunk(g, cl, v_b, sT_b, qTe, qTo):
            i = g * GCH + cl
            plt = psm2.tile([V, C], F32, tag="p2", name=f"plt{i}")
            nc.tensor.matmul(plt[:], v_b[:, ts(cl, V)], sT_b[:, ts(cl, C)],
                             start=True, stop=False)
            nc.tensor.matmul(plt[:], geo_bf[:, i * 2 * V : i * 2 * V + V],
                             qTe[:, ts(cl, C)], start=False, stop=False)
            nc.tensor.matmul(plt[:], geo_bf[:, i * 2 * V + V : (i + 1) * 2 * V],
                             qTo[:, ts(cl, C)], start=False, stop=True)
            if cl % 2 == 0:
                nc.scalar.copy(lt_all[:, ts(i, C)], plt[:])
            else:
                nc.vector.tensor_copy(lt_all[:, ts(i, C)], plt[:])
            out_sb = opool.tile([C, E], BF16, tag="osb", name=f"out_sb{i}")
            for h in range(2):
                po = pmain.tile([C, GW], F32, tag="pM", name=f"po{i}_{h}")
                nc.tensor.matmul(po[:], lt_all[:, ts(i, C)],
                                 wo2[0:64, ts(h, GW)], start=True, stop=True)
                dst = out_sb[:, ts(h, GW)]
                if h == 0:
                    nc.scalar.copy(dst, po[:])
                else:
                    nc.vector.tensor_copy(dst, po[:])
            nc.sync.dma_start(out_d[i], out_sb[:])

        # ============ PE warm-up ============
        # ~35 filler matmuls on a memset scratch while the header streams
        # in: HAM flips to K=8/8 (~3.4us of sustained PE busy) so the real
        # matmul stream runs at 2.4 GHz from the start. The dummies write
        # the prefix PSUM tile, which the real chain clears via start=True.
        scr = consts.tile([128, C], BF16, name="warm_scr")
        nc.vector.memset(scr[:], 0.0)
        pkv1 = psm2.tile([2 * K, PRE], F32, tag="p2", name="pkv1")
        for _ in range(50):
            nc.tensor.matmul(pkv1[:], scr[:], scr[:], start=True, stop=True)

        # ============ prefix projection + state (1 chunk) ============
        for ec in range(NEC):
            nc.tensor.matmul(pkv1[:], wkv[:, ts(ec, 2 * K)], xp[:, ts(ec, PRE)],
                             start=(ec == 0), stop=(ec == NEC - 1))
        kT1 = spool.tile([K, PRE], BF16, tag="kT1", name="kT1")
        nc.scalar.activation(kT1[:], pkv1[0:K, :], SIG, bias=bk_ap)
        vT1 = spool.tile([2 * K, PRE], BF16, tag="vT1", name="vT1")
        nc.vector.tensor_copy(vT1[K : 2 * K, :], pkv1[K : 2 * K, :])
        if has_bv:
            nc.vector.tensor_scalar_add(vT1[K : 2 * K, :], vT1[K : 2 * K, :], bv_ap)

        kn1, v1_b, veo1 = transposes(kT1, vT1, 1, "1")
        pu1 = psm2.tile([K, 2 * V], F32, tag="p2", name="pu1")
        nc.tensor.matmul(pu1[:, 0:V], kn1[:], veo1[:, 0:V], start=True, stop=True)
        nc.tensor.matmul(pu1[:, V : 2 * V], kn1[:], veo1[:, V : 2 * V],
                         start=True, stop=True)
        nc.vector.tensor_scalar_mul(geo_bf[:, ts2(0)], pu1[:],
                                    gamma[0:K, G_GAMMA : G_GAMMA + 1])

        # ============ group projections (PE chases the x stream) ============
        vT_sbs = []

        def proj_group(g):
            pkv = pmain.tile([2 * K, GW], F32, tag="pM", name=f"pkv_{g}")
            pqg = pmain.tile([K, GW], F32, tag="pM", name=f"pq_{g}")
            for hf in range(2):
                xh = xg2s[g][:, hf * HCOL : (hf + 1) * HCOL]
                dst = slice(hf * HW2, hf * HW2 + HW2)
                for ec in range(NEC):
                    nc.tensor.matmul(pkv[0 : 2 * K, dst], wkv[:, ts(ec, 2 * K)],
                                     xh[:, ts(ec, HW2)],
                                     start=(ec == 0), stop=(ec == NEC - 1))
                for ec in range(NEC):
                    nc.tensor.matmul(pqg[0:K, dst], wq[:, ts(ec, K)],
                                     xh[:, ts(ec, HW2)],
                                     start=(ec == 0), stop=(ec == NEC - 1))
            nc.scalar.activation(kT_all[:, ts(g, GW)], pkv[0:K, :], SIG, bias=bk_ap)
            vT_sb = spool.tile([2 * K, GW], BF16, tag=f"vT{g}", name=f"vT_sb{g}")
            nc.vector.tensor_copy(vT_sb[K : 2 * K, :], pkv[K : 2 * K, :])
            if has_bv:
                nc.vector.tensor_scalar_add(
                    vT_sb[K : 2 * K, :], vT_sb[K : 2 * K, :], bv_ap)
            vT_sbs.append(vT_sb)
            nc.scalar.activation(qT_all[:, ts(g, GW)], pqg[:], SIG, bias=bq_ap)

        proj_group(0)

        pu2 = pstate.tile([K, (NCH - 1) * 2 * V], F32, name="pu2")

        # stage2 g0: transposes, scores, state updates, chain 1-3
        kn_0, v_0, veo_0 = transposes(kT_all[:, 0:GW], vT_sbs[0], GCH, "20")
        sT_0, qTe_0, qTo_0 = scores_stage(0)
        state_stage(0, kn_0, veo_0)
        for i in range(1, GCH):
            chain_step(i)

        proj_group(1)

        # stage2 g1 part A: transposes + scores (ring-ordered after proj g1)
        kn_1, v_1, veo_1 = transposes(kT_all[:, GW : 2 * GW], vT_sbs[1], GCH, "21")
        sT_1, qTe_1, qTo_1 = scores_stage(1)
        state_stage(1, kn_1, veo_1)

        # attn + out, group 0
        for cl in range(GCH):
            attn_out_chunk(0, cl, v_0, sT_0, qTe_0, qTo_0)

        # chain into group 1, attn + out group 1
        for cl in range(GCH):
            chain_step(GCH + cl)
            attn_out_chunk(1, cl, v_1, sT_1, qTe_1, qTo_1)

    nc.compile()
    return nc


_CACHE = {}


def _get_program(has_bv):
    key = ("nc", has_bv)
    if key not in _CACHE:
        _CACHE[key] = _build_program(has_bv)
    return _CACHE[key]


def _make_in_maps(x, Wk, bk, Wv, bv, Wq, bq, Wo):
    import ml_dtypes

    bfd = ml_dtypes.bfloat16
    consts = _host_constants()

    def pack2(Wa, Wb):
        # [128, NEC*(outA+outB)]: per embed sub-chunk, [Wa_ec | Wb_ec] columns
        Wab = np.concatenate(
            [Wa.T.reshape(NEC, 128, -1), Wb.T.reshape(NEC, 128, -1)], 2
        )
        return np.ascontiguousarray(
            Wab.transpose(1, 0, 2).reshape(128, -1)
        )

    def pack1(W):
        return np.ascontiguousarray(
            W.T.reshape(NEC, 128, -1).transpose(1, 0, 2).reshape(128, -1)
        )

    identhi = np.zeros((128, 64), np.float32)
    identhi[64:128, :] = np.eye(64)

    def setreg(pack, layout, name, arr):
        r, o, c = layout[name]
        pack[0:r, o : o + c] = arr

    cwb = np.zeros((K, CWB_W), np.float32)
    setreg(cwb, _CWB, "cemat", consts["cemat"])
    setreg(cwb, _CWB, "comat", consts["comat"])

    gcol = np.zeros((128, G_NCOL), np.float32)
    gcol[0:K, G_BKV] = bk
    gcol[K : 2 * K, G_BKV] = bv
    gcol[0:K, G_BQ] = bq
    gcol[:, G_WGE] = consts["wge"]
    gcol[:, G_WGO] = consts["wgo"]

    hdr = np.zeros((128, HDR_W), np.float32)
    setreg(hdr, _HDR, "wkv", pack2(Wk, Wv))
    setreg(hdr, _HDR, "wq", pack1(Wq))
    setreg(hdr, _HDR, "ident", consts["ident64"])
    setreg(hdr, _HDR, "identhi", identhi)
    wrep = np.concatenate([
        np.tile(consts["wge"][:, None], (1, GCH * V)),
        np.tile(consts["wgo"][:, None], (1, GCH * V)),
    ], 1)
    setreg(hdr, _HDR, "wrep", wrep)

    shared = {
        "mloc": consts["mlocT4"].astype(bfd),
        "cwb": cwb.astype(bfd),
        "wo2": np.ascontiguousarray(Wo.T).astype(bfd),
    }

    def pack_x(xh):
        # [E, HT] -> [NG*2, 128, (ec, t)]: one contiguous block per (g, half)
        v = xh.reshape(NEC, 128, NG, 2, HW2).transpose(2, 3, 1, 0, 4)
        return np.ascontiguousarray(v.reshape(NG * 2, 128, HCOL)).astype(bfd)

    def pack_pre(xh):
        # last PRE prefix rows -> [128, (ec, t)] contiguous block
        v = xh[:, HT - PRE :].reshape(NEC, 128, PRE).transpose(1, 0, 2)
        return np.ascontiguousarray(v.reshape(128, NEC * PRE))

    zeros_pre = np.zeros((128, NEC * PRE), bfd)
    in_maps = []
    for c in range(N_CORES):
        b, h = c // 2, c % 2
        xbT = np.ascontiguousarray(x[b].T)  # [E, T]
        m = dict(shared)
        hh = hdr.copy()
        g = gcol.copy()
        g[:, G_GAMMA] = float(h)
        setreg(hh, _HDR, "gamma", g)
        m["hdr"] = hh.astype(bfd)
        m["xpre"] = (pack_pre(xbT[:, :HT]).astype(bfd) if h == 1 else zeros_pre)
        m["xq4"] = pack_x(xbT[:, h * HT : (h + 1) * HT])
        in_maps.append(m)
    return in_maps


def run(inputs, trace=False):
    """Run on 8 cores; returns (output, BassKernelResults)."""
    inp = {k: np.asarray(v) for k, v in inputs.items()}
    has_bv = bool(np.any(inp["bv"]))
    nc = _get_program(has_bv)
    in_maps = _make_in_maps(**inp)
    res = bass_utils.run_bass_kernel_spmd(
        nc, in_maps, core_ids=list(range(N_CORES)), trace=trace
    )
    out = np.empty((B, T, E), np.float32)
    for c in range(N_CORES):
        b, h = c // 2, c % 2
        o = res.results[c]["out"].astype(np.float32)  # [NCH, 128, E]
        out[b, h * HT : (h + 1) * HT, :] = o.reshape(HT, E)
    return out, res


def kernel(**inputs):
    out, _ = run(inputs, trace=False)
    return out


# revision 53
# speedup vs baseline: 1.0506x; 1.0197x over previous
"""Trainium2 Bass kernel for nn_Decay2D (decay-masked linear attention).

Math: the reference's Hillis-Steele scan with decay-squaring order composes
to coefficient d^ceil((t-s)/2) on store[s] = scale*k_s v_s^T, so

    out[t] = scale^2 * sum_{s<=t} d^ceil((t-s)/2) (q_t . k_s) v_s  @ Wo^T

computed as chunked linear attention with a [K, 2V] carry state per chunk
(even/odd decay chains on the V axis), never materializing [B,T,K,V].

Sharding: 8 cores = 4 batches x 2 sequence halves. Each core builds the
carry state over a truncated 128-row prefix and runs full attention +
output projection for its own 1024 rows.

v11 (~44.5us vs 50.2us v3 baseline): the body is built around the DMA
stream. All transfers are contiguous DRAM blocks (header with
gamma/weights/idents/wrep, prefix block, per-(group,half) x blocks, mloc,
a 64-row ce/co/Wo block, per-chunk output stores) on the sync HWDGE ring
in consumption order; the PE chases the stream. ~50 warm-up matmuls on a
memset scratch run while the header streams in so HAM reaches K=8/8
before real work. PSUM rings are split so the attention accumulators
(plt) ping-pong independently of the projection/score/out-proj ring. The
carry chain is bf16 end-to-end (one DVE stt per chunk, no casts), and the
parity weights ride a header-shipped broadcast block (wrep) because both
DVE and POOL tensor_scalar are slow paths (~4x and ~13x vs tensor_mul).
"""

from contextlib import ExitStack

import numpy as np

import concourse.bass as bass
import concourse.bacc as bacc
import concourse.mybir as mybir
import concourse.tile as tile
from concourse import bass_utils
from concourse.alu_op_type import AluOpType
from concourse.bass import ts

F32 = mybir.dt.float32
BF16 = mybir.dt.bfloat16
SIG = mybir.ActivationFunctionType.Sigmoid

B, T, E, K, V = 4, 2048, 1024, 64, 64
DECAY = 0.9
C = 128          # chunk length
HT = T // 2      # rows per core (sequence half)
NCH = HT // C    # chunks per half (8)
NEC = E // 128   # embed sub-chunks (8)
GW = 512         # group width: 4 chunks per PSUM bank
GCH = GW // C    # chunks per group (4)
NG = HT // GW    # groups per half (2)
HW2 = GW // 2    # half-group width (256)
HCOL = NEC * HW2 # x columns per (group, half) block (2048)
DC2 = float(DECAY ** (C // 2))
N_CORES = 8
PRE = 128        # truncated prefix length (1 chunk; older rows decay < 2e-3)

def _mklayout(regions):
    out, off = {}, 0
    for n, r, c in regions:
        out[n] = (r, off, c)
        off += c
    return out, off


# gamma columns (bf16, inside hdr): 0 prefix flag, 1 bk|bv, 2 bq, 3 wge, 4 wgo
G_GAMMA, G_BKV, G_BQ, G_WGE, G_WGO = 0, 1, 2, 3, 4
G_NCOL = 8

_HDR, HDR_W = _mklayout([
    ("gamma", 128, G_NCOL),
    ("wkv", 128, NEC * 2 * K), ("wq", 128, NEC * K),
    ("ident", 64, 64), ("identhi", 128, 64),
    ("wrep", 128, 2 * GCH * V),
])
_CWB, CWB_W = _mklayout([
    ("cemat", K, GW), ("comat", K, GW),
])


def _host_constants():
    d = DECAY
    scale2 = 1.0 - d
    i = np.arange(C)
    j = np.arange(C)
    delta = i[:, None] - j[None, :]
    # intra-chunk decay mask, transposed to [tcol(j), trow(i)], scale^2 folded
    mloc = np.where(delta >= 0, d ** np.ceil(delta / 2.0), 0.0) * scale2
    mlocT4 = np.tile(np.ascontiguousarray(mloc.T), (1, GCH)).astype(np.float32)
    # boundary coefficient per local row i (scale^2 folded), split by parity
    c = d ** np.ceil((i + 1) / 2.0) * scale2
    ce = np.where(i % 2 == 0, c, 0.0).astype(np.float32)
    co = np.where(i % 2 == 1, c, 0.0).astype(np.float32)
    cemat = np.tile(np.broadcast_to(ce, (K, C)), (1, GCH)).astype(np.float32)
    comat = np.tile(np.broadcast_to(co, (K, C)), (1, GCH)).astype(np.float32)
    # state-update row weights (per t within chunk)
    u_o = np.where(j % 2 == 1, d ** ((C - 1 - j) / 2.0), 0.0)
    u_e = np.where(j % 2 == 0, d ** ((C - 2 - j) / 2.0), 0.0)
    wge = (u_o + u_e).astype(np.float32)          # [C]
    wgo = (u_o + d * u_e).astype(np.float32)
    return {
        "mlocT4": mlocT4,
        "cemat": np.ascontiguousarray(cemat),
        "comat": np.ascontiguousarray(comat),
        "wge": wge,
        "wgo": wgo,
        "ident64": np.eye(64, dtype=np.float32),
    }


def _build_program(has_bv):
    nc = bacc.Bacc(
        "TRN2",
        debug=False,
        enable_asserts=False,
        target_bir_lowering=False,
        num_devices=N_CORES,
    )

    def din(name, shape, dtype=BF16):
        return nc.dram_tensor(name, shape, dtype, kind="ExternalInput").ap()

    hdr_d = din("hdr", [128, HDR_W])
    xpre_d = din("xpre", [128, NEC * PRE])        # prefix x, (ec, t)
    xq4 = din("xq4", [NG * 2, 128, HCOL])         # x per (group, half), (ec, t)
    mloc_d = din("mloc", [C, GW])
    cwb_d = din("cwb", [K, CWB_W])                # ce / co (64 rows)
    wo2_d = din("wo2", [K, E])                    # Wo^T
    out_d = nc.dram_tensor("out", [NCH, 128, E], BF16,
                           kind="ExternalOutput").ap()

    with ExitStack() as ctx:
        tc = ctx.enter_context(tile.TileContext(nc))

        consts = ctx.enter_context(tc.tile_pool(name="consts", bufs=1))
        state = ctx.enter_context(tc.tile_pool(name="state", bufs=1))
        xpool = ctx.enter_context(tc.tile_pool(name="xg", bufs=2))
        spool = ctx.enter_context(tc.tile_pool(name="sml", bufs=2))
        opool = ctx.enter_context(tc.tile_pool(name="osb", bufs=3))
        # PSUM budget (8 banks): pmain ring 2 (pkv/pq/ps/po), psm2 ring 2
        # (pkv1/pu1/plt ping-pong), psml ring 2 (transposes), pstate 2 (pu2)
        pmain = ctx.enter_context(tc.tile_pool(name="pmain", bufs=2, space="PSUM"))
        psm2 = ctx.enter_context(tc.tile_pool(name="psm2", bufs=2, space="PSUM"))
        psml = ctx.enter_context(tc.tile_pool(name="psml", bufs=2, space="PSUM"))
        pstate = ctx.enter_context(tc.tile_pool(name="pstate", bufs=1, space="PSUM"))

        # ---- loads in wire order (single sync HWDGE ring = FIFO) ----
        hdr = consts.tile([128, HDR_W], BF16, name="hdr")
        nc.sync.dma_start(hdr[:], hdr_d[:])
        xp = consts.tile([128, NEC * PRE], BF16, name="xp")
        nc.sync.dma_start(xp[:], xpre_d[:])

        xg2s = [xpool.tile([128, 2 * HCOL], BF16, tag="xg", name=f"xg2_{g}")
                for g in range(NG)]

        def load_xhalf(g, hf):
            nc.sync.dma_start(
                xg2s[g][:, hf * HCOL : (hf + 1) * HCOL], xq4[g * 2 + hf])

        load_xhalf(0, 0)
        load_xhalf(0, 1)
        mlocT4 = consts.tile([C, GW], BF16, name="mloc")
        nc.sync.dma_start(mlocT4[:], mloc_d[:])
        cwb = consts.tile([K, CWB_W], BF16, name="cwb")
        nc.sync.dma_start(cwb[:], cwb_d[:])
        wo2 = consts.tile([K, E], BF16, name="wo2")
        nc.sync.dma_start(wo2[:], wo2_d[:])
        load_xhalf(1, 0)
        load_xhalf(1, 1)

        def reg(pack, layout, name):
            r, o, c = layout[name]
            return pack[0:r, o : o + c]

        gamma = consts.tile([128, G_NCOL], F32, name="gamma_f32")
        nc.vector.tensor_copy(gamma[:], reg(hdr, _HDR, "gamma"))
        wkv, wq = reg(hdr, _HDR, "wkv"), reg(hdr, _HDR, "wq")
        ident, identhi = reg(hdr, _HDR, "ident"), reg(hdr, _HDR, "identhi")
        cemat, comat = reg(cwb, _CWB, "cemat"), reg(cwb, _CWB, "comat")
        bk_ap = gamma[0:K, G_BKV : G_BKV + 1]
        bv_ap = gamma[K : 2 * K, G_BKV : G_BKV + 1]
        bq_ap = gamma[0:K, G_BQ : G_BQ + 1]
        wge_ap = gamma[:, G_WGE : G_WGE + 1]
        wgo_ap = gamma[:, G_WGO : G_WGO + 1]

        qT_all = consts.tile([K, HT], BF16, name="qT_all")
        kT_all = consts.tile([K, HT], BF16, name="kT_all")
        lt_all = consts.tile([V, HT], BF16, name="lt_all")
        # carry state: parity on the V axis -> [K, NCH * 2V], kept in bf16
        # (the 7-step chain loses ~0.3% which is well inside tolerance)
        geo_bf = state.tile([K, NCH * 2 * V], BF16, name="geo_bf")
        # wge/wgo broadcast into [C, GCH*V] blocks, shipped in the header
        wrep = reg(hdr, _HDR, "wrep")

        def ts2(i):  # [K, 2V] slice of the state for chunk i
            return slice(i * 2 * V, (i + 1) * 2 * V)

        # ---- stage helpers ----
        def transposes(kT_src, vT_src, nch, tagp):
            pkn = psml.tile([C, nch * K], BF16, tag="pS", name=f"pkn{tagp}")
            for cl in range(nch):
                nc.tensor.matmul(pkn[:, ts(cl, K)], kT_src[:, ts(cl, C)],
                                 ident[:], is_transpose=True)
            pvn = psml.tile([C, nch * V], BF16, tag="pS", name=f"pvn{tagp}")
            for cl in range(nch):
                nc.tensor.matmul(pvn[:, ts(cl, V)], vT_src[K : 2 * K, ts(cl, C)],
                                 identhi[K : 2 * K, :], is_transpose=True)
            kn = spool.tile([C, nch * K], BF16, tag=f"kn{tagp}", name=f"kn{tagp}")
            nc.scalar.copy(kn[:], pkn[:])
            v_b = spool.tile([C, nch * V], BF16, tag=f"v{tagp}", name=f"v{tagp}")
            nc.vector.tensor_copy(v_b[:], pvn[:])
            # parity-weighted v for the state update
            veo = spool.tile([C, 2 * nch * V], BF16, tag=f"veo{tagp}",
                             name=f"veo{tagp}")
            nc.vector.tensor_mul(veo[:, 0 : nch * V], v_b[:],
                                 wrep[:, 0 : nch * V])
            nc.vector.tensor_mul(veo[:, nch * V :], v_b[:],
                                 wrep[:, GCH * V : (GCH + nch) * V])
            return kn, v_b, veo

        def scores_stage(g):
            ps = pmain.tile([C, GW], F32, tag="pM", name=f"ps{g}")
            for cl in range(GCH):
                i = g * GCH + cl
                nc.tensor.matmul(ps[:, ts(cl, C)], kT_all[:, ts(i, C)],
                                 qT_all[:, ts(i, C)], start=True, stop=True)
            sT_b = spool.tile([C, GW], BF16, tag=f"sm{g}", name="sT_b")
            nc.vector.tensor_mul(sT_b[:], ps[:], mlocT4[:])
            qTe = spool.tile([K, GW], BF16, tag=f"qe{g}", name="qTe")
            nc.vector.tensor_mul(qTe[:], qT_all[:, ts(g, GW)], cemat[:])
            qTo = spool.tile([K, GW], BF16, tag=f"qo{g}", name="qTo")
            nc.gpsimd.tensor_mul(qTo[:], qT_all[:, ts(g, GW)], comat[:])
            return sT_b, qTe, qTo

        def state_stage(g, kn_g, veo_g):
            # pu2 updates for this group's chunks (skip the last chunk)
            for cl in range(GCH):
                i = g * GCH + cl
                if i < NCH - 1:
                    nc.tensor.matmul(pu2[:, i * 2 * V : i * 2 * V + V],
                                     kn_g[:, ts(cl, K)],
                                     veo_g[:, ts(cl, V)],
                                     start=True, stop=True)
                    nc.tensor.matmul(pu2[:, i * 2 * V + V : (i + 1) * 2 * V],
                                     kn_g[:, ts(cl, K)],
                                     veo_g[:, (GCH + cl) * V : (GCH + cl + 1) * V],
                                     start=True, stop=True)

        def chain_step(i):
            nc.vector.scalar_tensor_tensor(
                geo_bf[:, ts2(i)], geo_bf[:, ts2(i - 1)], DC2,
                pu2[:, ts2(i - 1)], AluOpType.mult, AluOpType.add,
            )

        def attn_out_chunk(g, cl, v_b, sT_b, qTe, qTo):
            i = g * GCH + cl
            plt = psm2.tile([V, C], F32, tag="p2", name=f"plt{i}")
            nc.tensor.matmul(plt[:], v_b[:, ts(cl, V)], sT_b[:, ts(cl, C)],
                             start=True, stop=False)
            nc.tensor.matmul(plt[:], geo_bf[:, i * 2 * V : i * 2 * V + V],
                             qTe[:, ts(cl, C)], start=False, stop=False)
            nc.tensor.matmul(plt[:], geo_bf[:, i * 2 * V + V : (i + 1) * 2 * V],
                             qTo[:, ts(cl, C)], start=False, stop=True)
            if cl % 2 == 0:
                nc.scalar.copy(lt_all[:, ts(i, C)], plt[:])
            else:
                nc.vector.tensor_copy(lt_all[:, ts(i, C)], plt[:])
            out_sb = opool.tile([C, E], BF16, tag="osb", name=f"out_sb{i}")
            for h in range(2):
                po = pmain.tile([C, GW], F32, tag="pM", name=f"po{i}_{h}")
                nc.tensor.matmul(po[:], lt_all[:, ts(i, C)],
                                 wo2[0:64, ts(h, GW)], start=True, stop=True)
                dst = out_sb[:, ts(h, GW)]
                if h == 0:
                    nc.scalar.copy(dst, po[:])
                else:
                    nc.vector.tensor_copy(dst, po[:])
            nc.sync.dma_start(out_d[i], out_sb[:])

        # ============ PE warm-up ============
        # ~35 filler matmuls on a memset scratch while the header streams
        # in: HAM flips to K=8/8 (~3.4us of sustained PE busy) so the real
        # matmul stream runs at 2.4 GHz from the start. The dummies write
        # the prefix PSUM tile, which the real chain clears via start=True.
        scr = consts.tile([128, C], BF16, name="warm_scr")
        nc.vector.memset(scr[:], 0.0)
        pkv1 = psm2.tile([2 * K, PRE], F32, tag="p2", name="pkv1")
        for _ in range(50):
            nc.tensor.matmul(pkv1[:], scr[:], scr[:], start=True, stop=True)

        # ============ prefix projection + state (1 chunk) ============
        for ec in range(NEC):
            nc.tensor.matmul(pkv1[:], wkv[:, ts(ec, 2 * K)], xp[:, ts(ec, PRE)],
                             start=(ec == 0), stop=(ec == NEC - 1))
        kT1 = spool.tile([K, PRE], BF16, tag="kT1", name="kT1")
        nc.scalar.activation(kT1[:], pkv1[0:K, :], SIG, bias=bk_ap)
        vT1 = spool.tile([2 * K, PRE], BF16, tag="vT1", name="vT1")
        nc.vector.tensor_copy(vT1[K : 2 * K, :], pkv1[K : 2 * K, :])
        if has_bv:
            nc.vector.tensor_scalar_add(vT1[K : 2 * K, :], vT1[K : 2 * K, :], bv_ap)

        kn1, v1_b, veo1 = transposes(kT1, vT1, 1, "1")
        pu1 = psm2.tile([K, 2 * V], F32, tag="p2", name="pu1")
        nc.tensor.matmul(pu1[:, 0:V], kn1[:], veo1[:, 0:V], start=True, stop=True)
        nc.tensor.matmul(pu1[:, V : 2 * V], kn1[:], veo1[:, V : 2 * V],
                         start=True, stop=True)
        nc.vector.tensor_scalar_mul(geo_bf[:, ts2(0)], pu1[:],
                                    gamma[0:K, G_GAMMA : G_GAMMA + 1])

        # ============ group projections (PE chases the x stream) ============
        vT_sbs = []

        def proj_group(g):
            pkv = pmain.tile([2 * K, GW], F32, tag="pM", name=f"pkv_{g}")
            pqg = pmain.tile([K, GW], F32, tag="pM", name=f"pq_{g}")
            for hf in range(2):
                xh = xg2s[g][:, hf * HCOL : (hf + 1) * HCOL]
                dst = slice(hf * HW2, hf * HW2 + HW2)
                for ec in range(NEC):
                    nc.tensor.matmul(pkv[0 : 2 * K, dst], wkv[:, ts(ec, 2 * K)],
                                     xh[:, ts(ec, HW2)],
                                     start=(ec == 0), stop=(ec == NEC - 1))
                for ec in range(NEC):
                    nc.tensor.matmul(pqg[0:K, dst], wq[:, ts(ec, K)],
                                     xh[:, ts(ec, HW2)],
                                     start=(ec == 0), stop=(ec == NEC - 1))
            nc.scalar.activation(kT_all[:, ts(g, GW)], pkv[0:K, :], SIG, bias=bk_ap)
            vT_sb = spool.tile([2 * K, GW], BF16, tag=f"vT{g}", name=f"vT_sb{g}")
            nc.vector.tensor_copy(vT_sb[K : 2 * K, :], pkv[K : 2 * K, :])
            if has_bv:
                nc.vector.tensor_scalar_add(
                    vT_sb[K : 2 * K, :], vT_sb[K : 2 * K, :], bv_ap)
            vT_sbs.append(vT_sb)
            nc.scalar.activation(qT_all[:, ts(g, GW)], pqg[:], SIG, bias=bq_ap)

        proj_group(0)

        pu2 = pstate.tile([K, (NCH - 1) * 2 * V], F32, name="pu2")

        # stage2 g0: transposes, scores, state updates, chain 1-3
        kn_0, v_0, veo_0 = transposes(kT_all[:, 0:GW], vT_sbs[0], GCH, "20")
        sT_0, qTe_0, qTo_0 = scores_stage(0)
        state_stage(0, kn_0, veo_0)
        for i in range(1, GCH):
            chain_step(i)

        proj_group(1)

        # stage2 g1 part A: transposes + scores (ring-ordered after proj g1)
        kn_1, v_1, veo_1 = transposes(kT_all[:, GW : 2 * GW], vT_sbs[1], GCH, "21")
        sT_1, qTe_1, qTo_1 = scores_stage(1)
        state_stage(1, kn_1, veo_1)

        # attn + out, group 0
        for cl in range(GCH):
            attn_out_chunk(0, cl, v_0, sT_0, qTe_0, qTo_0)

        # chain into group 1, attn + out group 1
        for cl in range(GCH):
            chain_step(GCH + cl)
            attn_out_chunk(1, cl, v_1, sT_1, qTe_1, qTo_1)

    nc.compile()
    return nc


_CACHE = {}


def _get_program(has_bv):
    key = ("nc", has_bv)
    if key not in _CACHE:
        _CACHE[key] = _build_program(has_bv)
    return _CACHE[key]


def _make_in_maps(x, Wk, bk, Wv, bv, Wq, bq, Wo):
    import ml_dtypes

    bfd = ml_dtypes.bfloat16
    consts = _host_constants()

    def pack2(Wa, Wb):
        # [128, NEC*(outA+outB)]: per embed sub-chunk, [Wa_ec | Wb_ec] columns
        Wab = np.concatenate(
            [Wa.T.reshape(NEC, 128, -1), Wb.T.reshape(NEC, 128, -1)], 2
        )
        return np.ascontiguousarray(
            Wab.transpose(1, 0, 2).reshape(128, -1)
        )

    def pack1(W):
        return np.ascontiguousarray(
            W.T.reshape(NEC, 128, -1).transpose(1, 0, 2).reshape(128, -1)
        )

    identhi = np.zeros((128, 64), np.float32)
    identhi[64:128, :] = np.eye(64)

    def setreg(pack, layout, name, arr):
        r, o, c = layout[name]
        pack[0:r, o : o + c] = arr

    cwb = np.zeros((K, CWB_W), np.float32)
    setreg(cwb, _CWB, "cemat", consts["cemat"])
    setreg(cwb, _CWB, "comat", consts["comat"])

    gcol = np.zeros((128, G_NCOL), np.float32)
    gcol[0:K, G_BKV] = bk
    gcol[K : 2 * K, G_BKV] = bv
    gcol[0:K, G_BQ] = bq
    gcol[:, G_WGE] = consts["wge"]
    gcol[:, G_WGO] = consts["wgo"]

    hdr = np.zeros((128, HDR_W), np.float32)
    setreg(hdr, _HDR, "wkv", pack2(Wk, Wv))
    setreg(hdr, _HDR, "wq", pack1(Wq))
    setreg(hdr, _HDR, "ident", consts["ident64"])
    setreg(hdr, _HDR, "identhi", identhi)
    wrep = np.concatenate([
        np.tile(consts["wge"][:, None], (1, GCH * V)),
        np.tile(consts["wgo"][:, None], (1, GCH * V)),
    ], 1)
    setreg(hdr, _HDR, "wrep", wrep)

    shared = {
        "mloc": consts["mlocT4"].astype(bfd),
        "cwb": cwb.astype(bfd),
        "wo2": np.ascontiguousarray(Wo.T).astype(bfd),
    }

    def pack_x(xh):
        # [E, HT] -> [NG*2, 128, (ec, t)]: one contiguous block per (g, half)
        v = xh.reshape(NEC, 128, NG, 2, HW2).transpose(2, 3, 1, 0, 4)
        return np.ascontiguousarray(v.reshape(NG * 2, 128, HCOL)).astype(bfd)

    def pack_pre(xh):
        # last PRE prefix rows -> [128, (ec, t)] contiguous block
        v = xh[:, HT - PRE :].reshape(NEC, 128, PRE).transpose(1, 0, 2)
        return np.ascontiguousarray(v.reshape(128, NEC * PRE))

    zeros_pre = np.zeros((128, NEC * PRE), bfd)
    in_maps = []
    for c in range(N_CORES):
        b, h = c // 2, c % 2
        xbT = np.ascontiguousarray(x[b].T)  # [E, T]
        m = dict(shared)
        hh = hdr.copy()
        g = gcol.copy()
        g[:, G_GAMMA] = float(h)
        setreg(hh, _HDR, "gamma", g)
        m["hdr"] = hh.astype(bfd)
        m["xpre"] = (pack_pre(xbT[:, :HT]).astype(bfd) if h == 1 else zeros_pre)
        m["xq4"] = pack_x(xbT[:, h * HT : (h + 1) * HT])
        in_maps.append(m)
    return in_maps


def run(inputs, trace=False):
    """Run on 8 cores; returns (output, BassKernelResults)."""
    inp = {k: np.asarray(v) for k, v in inputs.items()}
    has_bv = bool(np.any(inp["bv"]))
    nc = _get_program(has_bv)
    in_maps = _make_in_maps(**inp)
    res = bass_utils.run_bass_kernel_spmd(
        nc, in_maps, core_ids=list(range(N_CORES)), trace=trace
    )
    out = np.empty((B, T, E), np.float32)
    for c in range(N_CORES):
        b, h = c // 2, c % 2
        o = res.results[c]["out"].astype(np.float32)  # [NCH, 128, E]
        out[b, h * HT : (h + 1) * HT, :] = o.reshape(HT, E)
    return out, res


def kernel(**inputs):
    out, _ = run(inputs, trace=False)
    return out
